# revision 30
# baseline (speedup 1.0000x reference)
"""Trainium2 Bass kernel for nn_DecoderTreeNN (gather + segment_sum over trees).

Computes, for two embedding tables C_hop / C_hop1:
    out[t, seg, :] = sum_{i : tree_ids[i] == seg} C_t[token_ids[i], :]
returning [2, 32, 512, 128] f32.

Strategy (8 NeuronCores, SPMD):
  - 16384 segments -> 128 "windows" of 128 consecutive segments. Core c owns
    windows [16c, 16c+16); since tree_ids is sorted, each window's tokens are
    a contiguous slice of the token stream. Host pads every window to a fixed
    16384 token slots; pad tokens use id 0, whose embedding row is all-zero
    (padding_idx), so they contribute nothing. Pads sit at the FRONT of the
    window so the sorted real tokens keep monotone gather addresses.
  - Host concatenates the two tables into one [32000, 256] bf16 table, so one
    gathered row (512 B) serves both outputs.
  - On device, per CHUNK-token chunk one gpsimd.dma_gather pulls the rows into
    SBUF as [128, NJ, 256] (token k = j*128 + p). Per chunk, ONE DVE
    tensor_tensor builds all NJ selection tiles S[p, j, s] =
    (tree_rel[p, j] == s) via broadcast APs (iota vs per-tile scalar); the PE
    accumulates S_j^T @ G_j -> PSUM[128 segs, 256] across the window's tiles.
  - PSUM is copied to SBUF (scalar engine) and DMA'd to a per-core
    [16, 128, 256] output; the host reassembles the full [2, 32, 512, 128].
"""

from contextlib import ExitStack

import ml_dtypes
import numpy as np

import concourse.bacc as bacc
import concourse.bass as bass
import concourse.mybir as mybir
import concourse.tile as tile
from concourse.bass_utils import run_bass_kernel_spmd
from concourse.library_config import mlp

P = 128
V = 32000
D = 128              # embedding dim per table
DD = 2 * D           # concatenated row width
N_CORES = 8
NSEG = 16384
SEGS_PW = 128        # segments per window
WG = NSEG // SEGS_PW             # 128 global windows
W = WG // N_CORES                # 16 windows per core
CHUNK = 2048                     # tokens per dma_gather
NCH = 7                          # chunks per window (pair-merged slots, was 8)
CAP = NCH * CHUNK                # padded slots per window (14336)
NJ = CHUNK // P                  # token tiles per chunk
NQ = W * NCH                     # chunks per core
SINGLE_PACKET = False            # dma_gather packetization mode (cap 1024 idxs)
GBUFS = 14                       # g-pool depth (gathers in flight)
SBUFS = 3                        # s-pool depth (DVE lookahead, chunks)
N_GSEMS = 16                     # >= GBUFS so no two in-flight gathers share a sem

_compiled = None


def _build_program(reps=1, mode="full", n_queues=4, sbufs=SBUFS, gbufs_n=GBUFS,
                   nj=NJ, nch=NCH, chunk=CHUNK, single_packet=SINGLE_PACKET):
    # mode: "full" | "gather_only" | "compute_only" | "contend" — probe modes
    # time sub-pipelines (outputs are garbage). "contend" runs gather and
    # compute concurrently with no cross edges (compute reads static tiles);
    # "contend_nos" additionally drops the DVE S-build (constant lhsT).
    contend = mode in ("contend", "contend_nos")
    no_s = mode == "contend_nos"
    do_gather = mode in (
        "full", "gather_only", "contend", "contend_nos", "gather_e256", "gather_e1024"
    )
    do_compute = mode in ("full", "compute_only", "contend", "contend_nos")
    # probe row widths: e256 gathers half-rows (256B) at full stride; e1024
    # gathers double-rows (1024B) from the pair-row view (idx values halved
    # host-side). Descriptor count is unchanged in both.
    elem = {"gather_e256": D, "gather_e1024": 2 * DD}.get(mode, DD)
    nq = W * nch
    nc = bacc.Bacc(
        "TRN2",
        target_bir_lowering=False,
        debug=False,
        num_devices=N_CORES,
        num_swdge_queues=n_queues,
    )
    t_table = nc.dram_tensor("table", [V, DD], mybir.dt.bfloat16, kind="ExternalInput")
    t_idx = nc.dram_tensor(
        "idx", [P, nq * (chunk // 16)], mybir.dt.int16, kind="ExternalInput"
    )
    t_trela = nc.dram_tensor(
        "trela", [P, nq * nj], mybir.dt.bfloat16, kind="ExternalInput"
    )
    t_trelb = nc.dram_tensor(
        "trelb", [P, nq * nj], mybir.dt.bfloat16, kind="ExternalInput"
    )
    t_cnt = nc.dram_tensor("cnt", [1, nq], mybir.dt.int32, kind="ExternalInput")
    t_iota = nc.dram_tensor("iota", [P, P], mybir.dt.bfloat16, kind="ExternalInput")
    t_out = nc.dram_tensor(
        "out", [reps * W, P, DD], mybir.dt.float32, kind="ExternalOutput"
    )

    with tile.TileContext(nc) as tc, ExitStack() as ctx:
        const = ctx.enter_context(tc.tile_pool(name="const", bufs=1))
        gpool = ctx.enter_context(tc.tile_pool(name="g", bufs=gbufs_n))
        spool = ctx.enter_context(tc.tile_pool(name="s", bufs=sbufs))
        sab = ctx.enter_context(tc.tile_pool(name="sab", bufs=4))
        opool = ctx.enter_context(tc.tile_pool(name="o", bufs=2))
        ppool = ctx.enter_context(tc.tile_pool(name="p", bufs=2, space="PSUM"))

        # One DMA sem per in-flight gather slot (rotating). N_GSEMS >= gbufs
        # guarantees the sem value 16*(q // N_GSEMS + 1) proves gather q is
        # fully drained on all 16 engines: the next user of the same sem
        # (gather q + N_GSEMS) cannot even be issued until gather q's
        # consumers ran (g-pool WAR), so no engine can contribute extra incs.
        assert N_GSEMS >= gbufs_n
        gsems = [nc.alloc_semaphore(f"gather_dma{i}") for i in range(N_GSEMS)]

        idx_all = const.tile([P, nq * (chunk // 16)], mybir.dt.int16)
        nc.sync.dma_start(idx_all[:], t_idx[:])
        cnt_all = const.tile([1, nq], mybir.dt.int32)
        nc.sync.dma_start(cnt_all[:], t_cnt[:])
        trela_all = const.tile([P, nq * nj], mybir.dt.bfloat16)
        nc.sync.dma_start(trela_all[:], t_trela[:])
        trelb_all = const.tile([P, nq * nj], mybir.dt.bfloat16)
        nc.sync.dma_start(trelb_all[:], t_trelb[:])
        iota_t = const.tile([P, P], mybir.dt.bfloat16)
        nc.sync.dma_start(iota_t[:], t_iota[:])

        nc.gpsimd.load_library(mlp)

        if not do_gather or contend:
            # compute_only/contend probe: static pre-zeroed g buffers
            gfix = [
                const.tile([P, nj, DD], mybir.dt.bfloat16, name=f"gfix{i}")
                for i in range(3)
            ]
            for gt in gfix:
                nc.vector.memset(gt[:], 0.0)

        gctr = 0
        if do_gather:
            # every chunk carries exactly `chunk` real descriptors (pads are
            # token 0 whose row is all-zero), so one shared count register
            # suffices (int-const num_idxs_reg is not a supported path)
            creg_const = nc.gpsimd.alloc_register("cnt_const")
            nc.gpsimd.reg_load(creg_const, cnt_all[0:1, 0:1])
        for r in range(reps):
            for w in range(W):
                psum = ppool.tile([P, DD], mybir.dt.float32, space="PSUM")
                for c in range(nch):
                    q = w * nch + c
                    if not do_gather:
                        g = gfix[gctr % 3]
                    else:
                        g = gpool.tile([P, nj, elem], mybir.dt.bfloat16, tag="g")
                    if contend:
                        g_dma, g = g, gfix[gctr % 3]
                    else:
                        g_dma = g
                    if do_gather:
                        idx_sl = idx_all[:, q * (chunk // 16) : (q + 1) * (chunk // 16)]
                        if mode == "gather_e256":
                            src = t_table[:, 0:D]
                            step = DD
                        elif mode == "gather_e1024":
                            src = t_table[:].rearrange("(a two) d -> a (two d)", two=2)
                            step = None
                        else:
                            src = t_table[:]
                            step = None
                        nc.gpsimd.dma_gather(
                            g_dma[:],
                            src,
                            idx_sl,
                            chunk,
                            creg_const,
                            elem,
                            elem_step=step,
                            # single-packet mode caps num_idxs at 16
                            # engines x 64 descs = 1024; beyond that the
                            # packet is malformed and wedges the device
                            single_packet=single_packet,
                            queue_num=gctr % n_queues,
                        ).then_inc(gsems[gctr % N_GSEMS], 16)
                    gctr += 1
                    if not do_compute:
                        continue
                    # three DVE ops build all nj selection tiles of this chunk:
                    # s[p, j, t] = (iota[t] == trela[p, q*nj+j])
                    #            + (iota[t] == trelb[p, q*nj+j])
                    # (each slot carries up to two occurrences of its vocab id
                    # — host pair-merged duplicates — so S rows can have two
                    # ones, or a two when both land in the same segment)
                    if not no_s:
                        iota_b = iota_t[:].unsqueeze(1).broadcast_to((P, nj, P))
                        sa = sab.tile([P, nj, P], mybir.dt.bfloat16, tag="sa")
                        nc.vector.tensor_tensor(
                            out=sa[:],
                            in0=iota_b,
                            in1=trela_all[:, q * nj : (q + 1) * nj]
                            .unsqueeze(2)
                            .broadcast_to((P, nj, P)),
                            op=mybir.AluOpType.is_equal,
                        )
                        sb = sab.tile([P, nj, P], mybir.dt.bfloat16, tag="sb")
                        nc.vector.tensor_tensor(
                            out=sb[:],
                            in0=iota_b,
                            in1=trelb_all[:, q * nj : (q + 1) * nj]
                            .unsqueeze(2)
                            .broadcast_to((P, nj, P)),
                            op=mybir.AluOpType.is_equal,
                        )
                        s = spool.tile([P, nj, P], mybir.dt.bfloat16, tag="s")
                        nc.vector.tensor_tensor(
                            out=s[:], in0=sa[:], in1=sb[:], op=mybir.AluOpType.add
                        )
                    for j in range(nj):
                        mm = nc.tensor.matmul(
                            out=psum[:],
                            lhsT=iota_t[:] if no_s else s[:, j, :],
                            rhs=g[:, j, :],
                            start=(c == 0 and j == 0),
                            stop=(c == nch - 1 and j == nj - 1),
                        )
                        if do_gather and not contend and j == 0:
                            mm._wait_ge(
                                gsems[(gctr - 1) % N_GSEMS],
                                16 * ((gctr - 1) // N_GSEMS + 1),
                            )
                if do_compute:
                    ot = opool.tile([P, DD], mybir.dt.float32, tag="o")
                    nc.scalar.copy(ot[:], psum[:])
                    nc.sync.dma_start(t_out[r * W + w], ot[:])
        if do_gather and (not do_compute or contend):
            # drain: every gather's sem must reach its final value before the
            # program ends (no matmul consumers exist to wait on them)
            total = reps * W * nch
            for i in range(N_GSEMS):
                n_i = total // N_GSEMS + (1 if i < total % N_GSEMS else 0)
                nc.gpsimd.wait_ge(gsems[i], 16 * n_i)

    nc.compile()
    return nc


def _col_layout(arr):
    # [WG, CAP] -> per-core [P, NQ*NJ]: column t = q*NJ + j, row p -> slot
    # k = j*128 + p of chunk q
    return np.ascontiguousarray(
        arr.reshape(N_CORES, W, NCH, NJ, P)
        .transpose(0, 4, 1, 2, 3)
        .reshape(N_CORES, P, NQ * NJ)
    )


def _pack_inputs(token_ids, tree_ids):
    tok = np.ascontiguousarray(np.asarray(token_ids, dtype=np.int32))
    tree = np.ascontiguousarray(np.asarray(tree_ids, dtype=np.int32))

    bounds = np.searchsorted(tree, np.arange(0, NSEG + 1, SEGS_PW))

    # Each slot holds one gathered row serving up to TWO occurrences of the
    # same vocab id (pair-merge dedup): segment sums are order-invariant, so
    # sorting a window's tokens by vocab id makes duplicates adjacent; pairs
    # (occurrence 2m, 2m+1) of a run share a slot via trela/trelb. Pad slots
    # (front): token 0 -> embedding row 0 is all-zero (padding_idx); trel -1
    # -> selection row is all-zero. Front padding keeps the sorted real
    # tokens' HBM gather addresses monotone (near-sequential).
    tok_pad = np.zeros((WG, CAP), dtype=np.int16)
    trela_pad = np.full((WG, CAP), -1.0, dtype=np.float32)
    trelb_pad = np.full((WG, CAP), -1.0, dtype=np.float32)
    for wg in range(WG):
        s, e = bounds[wg], bounds[wg + 1]
        n = e - s
        if n == 0:
            continue
        order = np.argsort(tok[s:e], kind="stable")
        ts = tok[s:e][order]
        rs = (tree[s:e][order] - SEGS_PW * wg).astype(np.float32)
        change = np.r_[True, ts[1:] != ts[:-1]]
        run_id = np.cumsum(change) - 1
        k = np.bincount(run_id)
        run_first = np.concatenate([[0], np.cumsum(k)[:-1]])
        occ = np.arange(n) - run_first[run_id]
        slots_before = np.concatenate([[0], np.cumsum((k + 1) // 2)[:-1]])
        slot = slots_before[run_id] + occ // 2
        nslots = int(slot[-1]) + 1 if n else 0
        assert nslots <= CAP, f"window {wg} overflow: {nslots} > {CAP}"
        base = CAP - nslots
        tok_pad[wg, base + slot] = ts.astype(np.int16)
        am = occ % 2 == 0
        trela_pad[wg, base + slot[am]] = rs[am]
        trelb_pad[wg, base + slot[~am]] = rs[~am]

    # idx: per chunk, index k lives at [16g + k%16, k//16], replicated g=0..7
    idx = (
        tok_pad.reshape(N_CORES, W, NCH, CHUNK // 16, 16)
        .transpose(0, 4, 1, 2, 3)
        .reshape(N_CORES, 16, NQ * (CHUNK // 16))
    )
    idx = np.broadcast_to(idx[:, None, :, :], (N_CORES, 8, 16, NQ * (CHUNK // 16)))
    idx = np.ascontiguousarray(idx.reshape(N_CORES, P, NQ * (CHUNK // 16)))

    trela = _col_layout(trela_pad).astype(ml_dtypes.bfloat16)
    trelb = _col_layout(trelb_pad).astype(ml_dtypes.bfloat16)
    # constant num_idxs: every chunk carries exactly CHUNK descriptors
    cnt = np.full((N_CORES, 1, NQ), CHUNK, dtype=np.int32)
    return idx, trela, trelb, cnt


def _prepare_in_maps(token_ids, tree_ids, C_hop, C_hop1):
    table = np.ascontiguousarray(
        np.concatenate(
            [np.asarray(C_hop, np.float32), np.asarray(C_hop1, np.float32)], axis=1
        ).astype(ml_dtypes.bfloat16)
    )
    idx, trela, trelb, cnt = _pack_inputs(token_ids, tree_ids)
    iota = np.ascontiguousarray(
        np.broadcast_to(
            np.arange(P, dtype=np.float32).astype(ml_dtypes.bfloat16), (P, P)
        )
    )
    return [
        {
            "table": table,
            "idx": idx[c],
            "trela": trela[c],
            "trelb": trelb[c],
            "cnt": cnt[c],
            "iota": iota,
        }
        for c in range(N_CORES)
    ]


def kernel(token_ids, tree_ids, C_hop, C_hop1, batch_size, max_trees):
    global _compiled
    batch_size = int(batch_size)
    max_trees = int(max_trees)
    assert batch_size * max_trees == NSEG

    in_maps = _prepare_in_maps(token_ids, tree_ids, C_hop, C_hop1)

    if _compiled is None:
        _compiled = _build_program()
    nc = _compiled
    res = run_bass_kernel_spmd(nc, in_maps, core_ids=list(range(N_CORES)))

    # assemble: res[c]["out"][w, s, :] = concat row for segment 2048c + 128w + s
    allseg = np.concatenate(
        [res.results[c]["out"].reshape(W * P, DD) for c in range(N_CORES)], axis=0
    )  # [16384, 256]
    key = allseg[:, :D].reshape(batch_size, max_trees, D)
    val = allseg[:, D:].reshape(batch_size, max_trees, D)
    return np.stack([key, val]).astype(np.float32)


# revision 57
# speedup vs baseline: 1.1506x; 1.1506x over previous
"""Trainium2 Bass kernel for nn_DecoderTreeNN (gather + segment_sum over trees).

Computes, for two embedding tables C_hop / C_hop1:
    out[t, seg, :] = sum_{i : tree_ids[i] == seg} C_t[token_ids[i], :]
returning [2, 32, 512, 128] f32.

Strategy (8 NeuronCores, SPMD):
  - 16384 segments -> 128 "windows" of 128 consecutive segments. Core c owns
    windows [16c, 16c+16); since tree_ids is sorted, each window's tokens are
    a contiguous slice of the token stream. Within a window, tokens are sorted
    by vocab id (segment sums are order-invariant) so gather addresses are
    monotone and duplicate ids become adjacent: pair-merge dedup gives each
    SLOT one gathered row serving up to two occurrences (trela/trelb segment
    ids; -1 = unused). Pad slots trail the window as token -1, which the
    gather ucode trims (per-window-last-chunk count register), so pads cost
    no descriptors.
  - Host concatenates the two tables into one [32000, 256] bf16 table, so one
    gathered row (512 B) serves both outputs.
  - On device, per CHUNK-slot chunk one gpsimd.dma_gather pulls the rows into
    SBUF as [128, NJ, 256] (slot k = j*128 + p). Per chunk, DVE tensor_tensor
    ops build selection tiles Sa/Sb[p, j, s] = (trel[p, j] == s) via broadcast
    APs (iota vs per-slot scalar); the PE accumulates Sa_j^T @ G_j (+ Sb_j^T
    @ G_j) -> PSUM[128 segs, 256] across the window's tiles.
  - PSUM is copied to SBUF (scalar engine) and DMA'd to a per-core
    [16, 128, 256] output; the host reassembles the full [2, 32, 512, 128].
"""

from contextlib import ExitStack

import ml_dtypes
import numpy as np

import concourse.bacc as bacc
import concourse.bass as bass
import concourse.mybir as mybir
import concourse.tile as tile
from concourse.bass_utils import run_bass_kernel_spmd
from concourse.library_config import mlp

P = 128
V = 32000
D = 128              # embedding dim per table
DD = 2 * D           # concatenated row width
N_CORES = 8
NSEG = 16384
SEGS_PW = 128        # segments per window
WG = NSEG // SEGS_PW             # 128 global windows
W = WG // N_CORES                # 16 windows per core
CHUNK = 2048                     # tokens per dma_gather
NCH = 7                          # chunks per window (pair-merged slots, was 8)
CAP = NCH * CHUNK                # padded slots per window (14336)
NJ = CHUNK // P                  # token tiles per chunk
NQ = W * NCH                     # chunks per core
SINGLE_PACKET = False            # dma_gather packetization mode (cap 1024 idxs)
GBUFS = 14                       # g-pool depth (gathers in flight)
SBUFS = 3                        # s-pool depth (DVE lookahead, chunks)
N_GSEMS = 16                     # >= GBUFS so no two in-flight gathers share a sem
# final configuration: pair-merge dedup, trailing-pad trim, pair slots
# clustered in each window's last B_CHUNKS chunks, PE-side accumulation
TRIM = True
B_CHUNKS = 2
S_ADD = False

_compiled = None


def _build_program(reps=1, mode="full", n_queues=4, sbufs=SBUFS, gbufs_n=GBUFS,
                   nj=NJ, nch=NCH, chunk=CHUNK, single_packet=SINGLE_PACKET,
                   use_b=True, s_add=S_ADD, trim=TRIM, b_chunks=B_CHUNKS):
    # use_b=False: no pair-merge (trelb ignored, single eq + single matmul;
    # pair with nch=8 inputs). s_add=False: keep sa/sb separate and let the
    # PE accumulate both into PSUM (two matmuls per tile, no DVE add pass).
    # trim=True: pads sit at the END of each window as token -1 and partial
    # chunks carry a per-chunk count register, so trailing pad descriptors
    # are never issued (pair with trim-packed inputs). b_chunks=N: only each
    # window's last N chunks hold pair-merged slots (host clusters them), so
    # earlier chunks run a single matmul per tile (requires trim, s_add=False).
    # mode: "full" | "gather_only" | "compute_only" | "contend" — probe modes
    # time sub-pipelines (outputs are garbage). "contend" runs gather and
    # compute concurrently with no cross edges (compute reads static tiles);
    # "contend_nos" additionally drops the DVE S-build (constant lhsT).
    contend = mode in ("contend", "contend_nos")
    no_s = mode == "contend_nos"
    do_gather = mode in (
        "full", "gather_only", "contend", "contend_nos", "gather_e256", "gather_e1024"
    )
    do_compute = mode in ("full", "compute_only", "contend", "contend_nos")
    # probe row widths: e256 gathers half-rows (256B) at full stride; e1024
    # gathers double-rows (1024B) from the pair-row view (idx values halved
    # host-side). Descriptor count is unchanged in both.
    elem = {"gather_e256": D, "gather_e1024": 2 * DD}.get(mode, DD)
    nq = W * nch
    nc = bacc.Bacc(
        "TRN2",
        target_bir_lowering=False,
        debug=False,
        num_devices=N_CORES,
        num_swdge_queues=n_queues,
    )
    t_table = nc.dram_tensor("table", [V, DD], mybir.dt.bfloat16, kind="ExternalInput")
    t_idx = nc.dram_tensor(
        "idx", [P, nq * (chunk // 16)], mybir.dt.int16, kind="ExternalInput"
    )
    t_trela = nc.dram_tensor(
        "trela", [P, nq * nj], mybir.dt.bfloat16, kind="ExternalInput"
    )
    t_trelb = nc.dram_tensor(
        "trelb", [P, nq * nj], mybir.dt.bfloat16, kind="ExternalInput"
    )
    t_cnt = nc.dram_tensor("cnt", [1, nq], mybir.dt.int32, kind="ExternalInput")
    t_iota = nc.dram_tensor("iota", [P, P], mybir.dt.bfloat16, kind="ExternalInput")
    t_out = nc.dram_tensor(
        "out", [reps * W, P, DD], mybir.dt.float32, kind="ExternalOutput"
    )

    with tile.TileContext(nc) as tc, ExitStack() as ctx:
        const = ctx.enter_context(tc.tile_pool(name="const", bufs=1))
        gpool = ctx.enter_context(tc.tile_pool(name="g", bufs=gbufs_n))
        spool = ctx.enter_context(tc.tile_pool(name="s", bufs=sbufs))
        sab = ctx.enter_context(tc.tile_pool(name="sab", bufs=4))
        opool = ctx.enter_context(tc.tile_pool(name="o", bufs=2))
        ppool = ctx.enter_context(tc.tile_pool(name="p", bufs=2, space="PSUM"))

        # One DMA sem per in-flight gather slot (rotating). N_GSEMS >= gbufs
        # guarantees the sem value 16*(q // N_GSEMS + 1) proves gather q is
        # fully drained on all 16 engines: the next user of the same sem
        # (gather q + N_GSEMS) cannot even be issued until gather q's
        # consumers ran (g-pool WAR), so no engine can contribute extra incs.
        assert N_GSEMS >= gbufs_n
        gsems = [nc.alloc_semaphore(f"gather_dma{i}") for i in range(N_GSEMS)]

        idx_all = const.tile([P, nq * (chunk // 16)], mybir.dt.int16)
        nc.sync.dma_start(idx_all[:], t_idx[:])
        cnt_all = const.tile([1, nq], mybir.dt.int32)
        nc.sync.dma_start(cnt_all[:], t_cnt[:])
        trela_all = const.tile([P, nq * nj], mybir.dt.bfloat16)
        nc.sync.dma_start(trela_all[:], t_trela[:])
        trelb_all = const.tile([P, nq * nj], mybir.dt.bfloat16)
        nc.sync.dma_start(trelb_all[:], t_trelb[:])
        iota_t = const.tile([P, P], mybir.dt.bfloat16)
        nc.sync.dma_start(iota_t[:], t_iota[:])

        nc.gpsimd.load_library(mlp)

        if not do_gather or contend:
            # compute_only/contend probe: static pre-zeroed g buffers
            gfix = [
                const.tile([P, nj, DD], mybir.dt.bfloat16, name=f"gfix{i}")
                for i in range(3)
            ]
            for gt in gfix:
                nc.vector.memset(gt[:], 0.0)

        gctr = 0
        if do_gather:
            # non-trim: every chunk carries exactly `chunk` real descriptors
            # (pads are token 0 whose row is all-zero), so one shared count
            # register suffices (int-const num_idxs_reg is not supported)
            creg_const = nc.gpsimd.alloc_register("cnt_const")
            nc.gpsimd.reg_load(creg_const, cnt_all[0:1, 0:1])
            if trim:
                # trimmed rows of a g slot are left untouched by the gather;
                # first rotation would otherwise read uninitialized SBUF,
                # and NaN garbage poisons the matmul (0 * NaN = NaN)
                for i in range(gbufs_n):
                    gz = gpool.tile([P, nj, DD], mybir.dt.bfloat16, tag="g")
                    nc.vector.memset(gz[:], 0.0)
                # the memset tiles are the pool's first rotation; gather into
                # them must wait for the memsets via the pool's WAW edges
            creg_trim = nc.gpsimd.alloc_register("cnt_trim") if trim else None
        for r in range(reps):
            for w in range(W):
                psum = ppool.tile([P, DD], mybir.dt.float32, space="PSUM")
                for c in range(nch):
                    q = w * nch + c
                    if not do_gather:
                        g = gfix[gctr % 3]
                    else:
                        g = gpool.tile([P, nj, elem], mybir.dt.bfloat16, tag="g")
                    if contend:
                        g_dma, g = g, gfix[gctr % 3]
                    else:
                        g_dma = g
                    if do_gather:
                        if trim and c >= nch - 1 - b_chunks:
                            # possibly-partial chunk: real slot count from cnt
                            # (trailing -1 pads are trimmed by the ucode and
                            # the ring reserves exactly this many descs)
                            nc.gpsimd.reg_load(creg_trim, cnt_all[0:1, q : q + 1])
                            creg = creg_trim
                        else:
                            creg = creg_const
                        idx_sl = idx_all[:, q * (chunk // 16) : (q + 1) * (chunk // 16)]
                        if mode == "gather_e256":
                            src = t_table[:, 0:D]
                            step = DD
                        elif mode == "gather_e1024":
                            src = t_table[:].rearrange("(a two) d -> a (two d)", two=2)
                            step = None
                        else:
                            src = t_table[:]
                            step = None
                        nc.gpsimd.dma_gather(
                            g_dma[:],
                            src,
                            idx_sl,
                            chunk,
                            creg,
                            elem,
                            elem_step=step,
                            # single-packet mode caps num_idxs at 16
                            # engines x 64 descs = 1024; beyond that the
                            # packet is malformed and wedges the device
                            single_packet=single_packet,
                            queue_num=gctr % n_queues,
                        ).then_inc(gsems[gctr % N_GSEMS], 16)
                    gctr += 1
                    if not do_compute:
                        continue
                    # three DVE ops build all nj selection tiles of this chunk:
                    # s[p, j, t] = (iota[t] == trela[p, q*nj+j])
                    #            + (iota[t] == trelb[p, q*nj+j])
                    # (each slot carries up to two occurrences of its vocab id
                    # — host pair-merged duplicates — so S rows can have two
                    # ones, or a two when both land in the same segment)
                    b_here = use_b and (not b_chunks or c >= nch - b_chunks)
                    if not no_s:
                        iota_b = iota_t[:].unsqueeze(1).broadcast_to((P, nj, P))
                        sa = sab.tile([P, nj, P], mybir.dt.bfloat16, tag="sa")
                        nc.vector.tensor_tensor(
                            out=sa[:],
                            in0=iota_b,
                            in1=trela_all[:, q * nj : (q + 1) * nj]
                            .unsqueeze(2)
                            .broadcast_to((P, nj, P)),
                            op=mybir.AluOpType.is_equal,
                        )
                        if b_here:
                            sb = sab.tile([P, nj, P], mybir.dt.bfloat16, tag="sb")
                            nc.vector.tensor_tensor(
                                out=sb[:],
                                in0=iota_b,
                                in1=trelb_all[:, q * nj : (q + 1) * nj]
                                .unsqueeze(2)
                                .broadcast_to((P, nj, P)),
                                op=mybir.AluOpType.is_equal,
                            )
                        if b_here and s_add:
                            s = spool.tile([P, nj, P], mybir.dt.bfloat16, tag="s")
                            nc.vector.tensor_tensor(
                                out=s[:], in0=sa[:], in1=sb[:], op=mybir.AluOpType.add
                            )
                            lhs_list = [s]
                        elif b_here:
                            lhs_list = [sa, sb]
                        else:
                            lhs_list = [sa]
                    for j in range(nj):
                        for li, lt in enumerate([None] if no_s else lhs_list):
                            mm = nc.tensor.matmul(
                                out=psum[:],
                                lhsT=iota_t[:] if no_s else lt[:, j, :],
                                rhs=g[:, j, :],
                                start=(c == 0 and j == 0 and li == 0),
                                stop=(
                                    c == nch - 1
                                    and j == nj - 1
                                    and li == (0 if no_s else len(lhs_list) - 1)
                                ),
                            )
                            if do_gather and not contend and j == 0 and li == 0:
                                mm._wait_ge(
                                    gsems[(gctr - 1) % N_GSEMS],
                                    16 * ((gctr - 1) // N_GSEMS + 1),
                                )
                if do_compute:
                    ot = opool.tile([P, DD], mybir.dt.float32, tag="o")
                    nc.scalar.copy(ot[:], psum[:])
                    nc.sync.dma_start(t_out[r * W + w], ot[:])
        if do_gather and (not do_compute or contend):
            # drain: every gather's sem must reach its final value before the
            # program ends (no matmul consumers exist to wait on them)
            total = reps * W * nch
            for i in range(N_GSEMS):
                n_i = total // N_GSEMS + (1 if i < total % N_GSEMS else 0)
                nc.gpsimd.wait_ge(gsems[i], 16 * n_i)

    nc.compile()
    return nc


def _col_layout(arr, nch):
    # [WG, cap] -> per-core [P, nq*NJ]: column t = q*NJ + j, row p -> slot
    # k = j*128 + p of chunk q
    return np.ascontiguousarray(
        arr.reshape(N_CORES, W, nch, NJ, P)
        .transpose(0, 4, 1, 2, 3)
        .reshape(N_CORES, P, W * nch * NJ)
    )


def _pack_inputs(token_ids, tree_ids, dedup=True, trim=TRIM, b_chunks=B_CHUNKS):
    assert not b_chunks or (dedup and trim)
    nch = NCH if dedup else 8
    cap = nch * CHUNK
    nq = W * nch
    tok = np.ascontiguousarray(np.asarray(token_ids, dtype=np.int32))
    tree = np.ascontiguousarray(np.asarray(tree_ids, dtype=np.int32))

    bounds = np.searchsorted(tree, np.arange(0, NSEG + 1, SEGS_PW))
    nslots_w = np.zeros(WG, dtype=np.int64)
    cnts_w = np.zeros((WG, nch), dtype=np.int64)

    # Each slot holds one gathered row serving up to TWO occurrences of the
    # same vocab id (pair-merge dedup): segment sums are order-invariant, so
    # sorting a window's tokens by vocab id makes duplicates adjacent; pairs
    # (occurrence 2m, 2m+1) of a run share a slot via trela/trelb. Pad slots
    # (front): token 0 -> embedding row 0 is all-zero (padding_idx); trel -1
    # -> selection row is all-zero. Front padding keeps the sorted real
    # tokens' HBM gather addresses monotone (near-sequential).
    tok_pad = np.full((WG, cap), -1 if trim else 0, dtype=np.int16)
    trela_pad = np.full((WG, cap), -1.0, dtype=np.float32)
    trelb_pad = np.full((WG, cap), -1.0, dtype=np.float32)
    for wg in range(WG):
        s, e = bounds[wg], bounds[wg + 1]
        n = e - s
        if n == 0:
            continue
        order = np.argsort(tok[s:e], kind="stable")
        ts = tok[s:e][order]
        rs = (tree[s:e][order] - SEGS_PW * wg).astype(np.float32)
        if dedup:
            change = np.r_[True, ts[1:] != ts[:-1]]
            run_id = np.cumsum(change) - 1
            k = np.bincount(run_id)
            run_first = np.concatenate([[0], np.cumsum(k)[:-1]])
            occ = np.arange(n) - run_first[run_id]
            slots_before = np.concatenate([[0], np.cumsum((k + 1) // 2)[:-1]])
            slot = slots_before[run_id] + occ // 2
            am = occ % 2 == 0
        else:
            slot = np.arange(n)
            am = np.ones(n, dtype=bool)
        nslots = int(slot[-1]) + 1
        assert nslots <= cap, f"window {wg} overflow: {nslots} > {cap}"
        nslots_w[wg] = nslots
        if b_chunks:
            # cluster pair-carrying slots into the last b_chunks chunks so
            # earlier chunks need only one matmul per tile; single-occupant
            # and pair slots each stay vocab-ascending (monotone HBM reads)
            has_b = np.bincount(slot, minlength=nslots) == 2
            a_cap = (nch - b_chunks) * CHUNK
            a_idx = np.flatnonzero(~has_b)
            b_idx = np.flatnonzero(has_b)
            n_a = len(a_idx)
            # the a-region gap must stay within the last a-chunk's tail, and
            # all pair slots must land inside the b-chunks
            assert n_a >= a_cap - CHUNK, f"window {wg}: a-region underflow"
            b_start = max(n_a, a_cap)
            assert b_start + len(b_idx) <= cap, f"window {wg}: b-region overflow"
            newpos = np.empty(nslots, dtype=np.int64)
            newpos[a_idx] = np.arange(n_a)
            newpos[b_idx] = b_start + np.arange(len(b_idx))
            pos = newpos[slot]
            occ_pos = np.zeros(cap, dtype=bool)
            occ_pos[:n_a] = True
            occ_pos[b_start : b_start + len(b_idx)] = True
            och = occ_pos.reshape(nch, CHUNK)
            # per-chunk real slots must be a prefix (pads strictly trailing)
            assert (np.diff(och.astype(np.int8), axis=1) <= 0).all()
            cnts_w[wg] = och.sum(axis=1)
            base = 0
        else:
            pos = slot
            # trim: slots at the front, -1 pads trail (trimmed, never
            # gathered); otherwise pads lead so real gather addresses stay
            # monotone
            base = 0 if trim else cap - nslots
            cnts_w[wg] = np.clip(nslots - np.arange(nch) * CHUNK, 0, CHUNK)
        tok_pad[wg, base + pos] = ts.astype(np.int16)
        trela_pad[wg, base + pos[am]] = rs[am]
        if not am.all():
            trelb_pad[wg, base + pos[~am]] = rs[~am]

    # idx: per chunk, index k lives at [16g + k%16, k//16], replicated g=0..7
    idx = (
        tok_pad.reshape(N_CORES, W, nch, CHUNK // 16, 16)
        .transpose(0, 4, 1, 2, 3)
        .reshape(N_CORES, 16, nq * (CHUNK // 16))
    )
    idx = np.broadcast_to(idx[:, None, :, :], (N_CORES, 8, 16, nq * (CHUNK // 16)))
    idx = np.ascontiguousarray(idx.reshape(N_CORES, P, nq * (CHUNK // 16)))

    trela = _col_layout(trela_pad, nch).astype(ml_dtypes.bfloat16)
    trelb = _col_layout(trelb_pad, nch).astype(ml_dtypes.bfloat16)
    if trim:
        # per-chunk real slot counts (partial chunks' -1 tails are trimmed)
        cnt = np.ascontiguousarray(
            cnts_w.reshape(N_CORES, 1, nq).astype(np.int32)
        )
    else:
        # constant num_idxs: every chunk carries exactly CHUNK descriptors
        cnt = np.full((N_CORES, 1, nq), CHUNK, dtype=np.int32)
    return idx, trela, trelb, cnt


def _prepare_in_maps(token_ids, tree_ids, C_hop, C_hop1, dedup=True, trim=TRIM,
                     b_chunks=B_CHUNKS):
    table = np.ascontiguousarray(
        np.concatenate(
            [np.asarray(C_hop, np.float32), np.asarray(C_hop1, np.float32)], axis=1
        ).astype(ml_dtypes.bfloat16)
    )
    idx, trela, trelb, cnt = _pack_inputs(
        token_ids, tree_ids, dedup=dedup, trim=trim, b_chunks=b_chunks
    )
    iota = np.ascontiguousarray(
        np.broadcast_to(
            np.arange(P, dtype=np.float32).astype(ml_dtypes.bfloat16), (P, P)
        )
    )
    return [
        {
            "table": table,
            "idx": idx[c],
            "trela": trela[c],
            "trelb": trelb[c],
            "cnt": cnt[c],
            "iota": iota,
        }
        for c in range(N_CORES)
    ]


def kernel(token_ids, tree_ids, C_hop, C_hop1, batch_size, max_trees):
    global _compiled
    batch_size = int(batch_size)
    max_trees = int(max_trees)
    assert batch_size * max_trees == NSEG

    in_maps = _prepare_in_maps(token_ids, tree_ids, C_hop, C_hop1)

    if _compiled is None:
        _compiled = _build_program()
    nc = _compiled
    res = run_bass_kernel_spmd(nc, in_maps, core_ids=list(range(N_CORES)))

    # assemble: res[c]["out"][w, s, :] = concat row for segment 2048c + 128w + s
    allseg = np.concatenate(
        [res.results[c]["out"].reshape(W * P, DD) for c in range(N_CORES)], axis=0
    )  # [16384, 256]
    key = allseg[:, :D].reshape(batch_size, max_trees, D)
    val = allseg[:, D:].reshape(batch_size, max_trees, D)
    return np.stack([key, val]).astype(np.float32)
